# revision 24
# baseline (speedup 1.0000x reference)
"""Trainium2 Bass kernel for nn_MultiHeadAttention (B=2, S=2048, D=1024, H=16).

Sharding: 8 cores = 2 batches x 4 head-groups (4 heads per core, tensor
parallel over heads). Each core computes, for its batch b and its 4 heads:
  QT/KT = (x @ W.T).T projections in transposed layout [256, 2048]
  V     = value @ wv.T in normal [token, dh] layout
  E^T   = exp(scoresT) tiles [k, q] straight from matmul (scores are O(1)
          here so exp without max-subtraction is safe; causal masking via a
          multiplicative 0/1 tile on the diagonal blocks)
  out   = (V^T E^T) / Z  with Z = colsum(E^T) from a ones-lhsT matmul
  ffT   = wff_rows-partial @ at -> [1024, 2048] bf16 partial, host-summed.

PE-array packing (tile_position derived from base partitions):
  scores: contraction is DH=64, so heads (h0,h1)/(h2,h3) run ROW-packed
          (two concurrent matmuls in array rows 0-63 / 64-127).
  AV:     output M=64 per head, so head pairs run COL-packed
          (array cols 0-63 / 64-127 -> psum partitions 0-63 / 64-127).
  Z:      4 concurrent M=32 matmuls (ones lhsT) at col positions
          0/32/64/96 -> zp psum rows fully covered, one reciprocal per
          q-group, gpsimd partition_broadcast for the per-head 1/Z rows.
A warm-up spinner issues dummy matmuls at t=0 so the PE HAM clock-gate
reaches 2.4 GHz before real work lands; inputs are host-prepacked into
group-major contiguous blocks so the first projection starts ~4us in.
"""

import sys

sys.path.insert(0, "/opt/trn_rl_repo")

import ml_dtypes
import numpy as np

import concourse.bass as bass
import concourse.mybir as mybir
import concourse.tile as tile
from concourse import bacc
from concourse import library_config
from concourse.bass_utils import run_bass_kernel_spmd

P = 128
B, S, D, H = 2, 2048, 1024, 16
DH = D // H  # 64
NCORES = 8
GPB = NCORES // B  # cores (head groups) per batch = 4
HPC = H // GPB  # heads per core = 4
HD = HPC * DH  # projected cols per core = 256
F32 = mybir.dt.float32
BF16 = mybir.dt.bfloat16
QGW = 512  # q-group width (psum free dim)
AF = mybir.ActivationFunctionType
NPBF16 = ml_dtypes.bfloat16

KC_N = D // P  # 8 contraction chunks over model dim
DC_N = HD // P  # 2 dout chunks
TG_N = S // QGW  # 4 token groups
TPG = QGW // P  # 4 token tiles per group

USE_GPSIMD_BCAST = True
NSPIN = 62

_PROG_CACHE: dict = {}


def build_causal(use_bias: bool):
    """Optimized causal-mask program with PE tile packing."""
    nc = bacc.Bacc("TRN2", target_bir_lowering=False, debug=False,
                   num_devices=NCORES)

    def din(name, shape, dt=BF16):
        return nc.dram_tensor(name, list(shape), dt, kind="ExternalInput").ap()

    xq_d = din("xq", (TG_N, P, KC_N, QGW))
    xk_d = din("xk", (TG_N, P, KC_N, QGW))
    xv_d = din("xv", (TG_N, P, KC_N, QGW))
    wq_d = din("wq", (P, KC_N, HD))   # pre-scaled by 1/sqrt(DH) on host
    wk_d = din("wk", (P, KC_N, HD))
    wv_d = din("wv", (P, KC_N, HD))
    wff_d = din("wff", (P, DC_N, D))
    dmask_d = din("dmask", (P, P))    # [k, q]: 1 if k <= q else 0
    if use_bias:
        bq_d = din("bq", (P, DC_N), F32)   # pre-scaled by 1/sqrt(DH)
        bk_d = din("bk", (P, DC_N), F32)
        bv_d = din("bv", (1, HD))
        bffq_d = din("bffq", (P, KC_N), F32)  # bff / GPB
        onesb_d = din("onesb", (1, P))
    outT = nc.dram_tensor("outT", [D, S], BF16, kind="ExternalOutput").ap()
    zdr = nc.dram_tensor("zdr", [TG_N * HPC, QGW], BF16).ap()

    with tile.TileContext(nc) as tc:
        with (
            nc.allow_low_precision(reason="bf16 matmul chain; psum stays f32"),
            tc.tile_pool(name="consts", bufs=1) as consts,
            tc.tile_pool(name="acts", bufs=1) as acts,
            tc.tile_pool(name="zpool", bufs=2) as zpool,
            tc.tile_pool(name="epool", bufs=6) as epool,
            tc.tile_pool(name="opool", bufs=4) as opool,
            tc.tile_pool(name="ps", bufs=1, space="PSUM") as ps,
        ):
            if USE_GPSIMD_BCAST:
                nc.gpsimd.load_library(library_config.attn)

            # ---- warm-up spinner (keeps PE busy so HAM un-throttles) ----
            spin = consts.tile([P, QGW], BF16, tag="spin")
            nc.vector.memset(spin[:], 0.0)
            onesz = consts.tile([P, 32], BF16, tag="onesz")
            nc.vector.memset(onesz[:], 1.0)
            spin_ps = ps.tile([P, QGW], F32, tag="op", bufs=2, name="spin_ps")
            for i in range(NSPIN):
                nc.tensor.matmul(spin_ps[:, :256], lhsT=spin[:, :P],
                                 rhs=spin[:, :256], start=True, stop=True)
            junk = consts.tile([P, 1], F32, tag="junk")
            nc.vector.tensor_copy(junk[:], spin_ps[:, 0:1])

            # ---- constant / weight / activation tiles ----
            dmask_sb = consts.tile([P, P], BF16, tag="dmask")
            wq_sb = consts.tile([P, KC_N, HD], BF16, tag="wq")
            wk_sb = consts.tile([P, KC_N, HD], BF16, tag="wk")
            wv_sb = consts.tile([P, KC_N, HD], BF16, tag="wv")
            wff_sb = consts.tile([P, DC_N, D], BF16, tag="wff")
            xq_g = [acts.tile([P, KC_N, QGW], BF16, tag=f"xq{g}")
                    for g in range(TG_N)]
            xk_g = [acts.tile([P, KC_N, QGW], BF16, tag=f"xk{g}")
                    for g in range(TG_N)]
            xv_g = [acts.tile([P, KC_N, QGW], BF16, tag=f"xv{g}")
                    for g in range(TG_N)]
            if use_bias:
                bq_sb = consts.tile([P, DC_N], F32, tag="bq")
                bk_sb = consts.tile([P, DC_N], F32, tag="bk")
                bv_sb = consts.tile([1, HD], BF16, tag="bv")
                bffq_sb = consts.tile([P, KC_N], F32, tag="bffq")
                onesb_sb = consts.tile([1, P], BF16, tag="onesb")

            # DMA emission order = start priority: group-0 activations
            # first, kc-halves separate so half-chains start early.
            nc.sync.dma_start(wq_sb[:], wq_d[:])
            nc.sync.dma_start(xq_g[0][:, 0:4, :], xq_d[0, :, 0:4, :])
            nc.sync.dma_start(wk_sb[:], wk_d[:])
            nc.sync.dma_start(xk_g[0][:, 0:4, :], xk_d[0, :, 0:4, :])
            nc.sync.dma_start(xq_g[0][:, 4:8, :], xq_d[0, :, 4:8, :])
            nc.sync.dma_start(xk_g[0][:, 4:8, :], xk_d[0, :, 4:8, :])
            nc.sync.dma_start(wv_sb[:], wv_d[:])
            nc.sync.dma_start(xv_g[0][:], xv_d[0])
            nc.sync.dma_start(dmask_sb[:], dmask_d[:])
            if use_bias:
                nc.sync.dma_start(bq_sb[:], bq_d[:])
                nc.sync.dma_start(bk_sb[:], bk_d[:])
                nc.sync.dma_start(bv_sb[:], bv_d[:])
                nc.sync.dma_start(bffq_sb[:], bffq_d[:])
                nc.sync.dma_start(onesb_sb[:], onesb_d[:])
            for g in (1, 2, 3):
                nc.sync.dma_start(xq_g[g][:], xq_d[g])
                nc.sync.dma_start(xk_g[g][:], xk_d[g])
                nc.sync.dma_start(xv_g[g][:], xv_d[g])
                if g == 1:
                    nc.sync.dma_start(wff_sb[:], wff_d[:])

            # per-group projected activations
            qT_g = [acts.tile([P, DC_N, QGW], BF16, tag=f"qT{g}")
                    for g in range(TG_N)]
            kT_g = [acts.tile([P, DC_N, QGW], BF16, tag=f"kT{g}")
                    for g in range(TG_N)]
            va_g = [acts.tile([P, TPG, HD], BF16, tag=f"va{g}")
                    for g in range(TG_N)]
            at_g = [acts.tile([P, DC_N, QGW], BF16, tag=f"at{g}")
                    for g in range(TG_N)]

            from collections import deque
            fillers = deque()

            def run_units(units, n=None):
                k = len(units) if n is None else min(n, len(units))
                for _ in range(k):
                    units.popleft()()

            def proj_qk_unit(tg, dc, w_sb, x_sb, b_sb, dest, ptag, pbufs,
                             half=None, cell=None):
                def unit(tg=tg, dc=dc, w_sb=w_sb, x_sb=x_sb, b_sb=b_sb,
                         dest=dest, ptag=ptag, pbufs=pbufs):
                    if half in (None, 0):
                        cell[0] = ps.tile([P, QGW], F32, tag=ptag, bufs=pbufs,
                                          name=f"pqk_{id(w_sb)}_{tg}_{dc}")
                    pp = cell[0]
                    k0 = 0 if half in (None, 0) else KC_N // 2
                    k1 = KC_N // 2 if half == 0 else KC_N
                    for kc in range(k0, k1):
                        nc.tensor.matmul(
                            pp[:],
                            lhsT=w_sb[:, kc, dc * P:(dc + 1) * P],
                            rhs=x_sb[:, kc, :],
                            start=(kc == 0),
                            stop=(kc == KC_N - 1),
                        )
                    if half == 0:
                        return
                    if use_bias:
                        nc.scalar.activation(dest[:, dc, :], pp[:],
                                             AF.Identity,
                                             bias=b_sb[:, dc:dc + 1])
                    else:
                        nc.vector.tensor_copy(dest[:, dc, :], pp[:])
                return unit

            def proj_v_unit(tg, kx, ptag, pbufs):
                def unit(tg=tg, kx=kx, ptag=ptag, pbufs=pbufs):
                    vp = ps.tile([P, QGW], F32, tag=ptag, bufs=pbufs,
                                 name=f"vp_{tg}_{kx}")
                    if use_bias:
                        nc.tensor.matmul(vp[:, :HD], lhsT=onesb_sb[0:1, :],
                                         rhs=bv_sb[:, :], start=True,
                                         stop=False)
                    for kc in range(KC_N):
                        nc.tensor.matmul(
                            vp[:, :HD],
                            lhsT=xv_g[tg][:, kc, kx * P:(kx + 1) * P],
                            rhs=wv_sb[:, kc, :],
                            start=(kc == 0 and not use_bias),
                            stop=(kc == KC_N - 1),
                        )
                    nc.vector.tensor_copy(va_g[tg][:, kx, :], vp[:, :HD])
                return unit

            def queue_proj_qk(tg, ptag="pacc", pbufs=1, halves=False):
                for w_sb, x_sb, b_sb, dest in (
                        (wq_sb, xq_g[tg], bq_sb if use_bias else None,
                         qT_g[tg]),
                        (wk_sb, xk_g[tg], bk_sb if use_bias else None,
                         kT_g[tg])):
                    for dc in range(DC_N):
                        if halves:
                            cell = {}
                            for half in range(2):
                                fillers.append(proj_qk_unit(
                                    tg, dc, w_sb, x_sb, b_sb, dest, ptag,
                                    pbufs, half=half, cell=cell))
                        else:
                            fillers.append(proj_qk_unit(
                                tg, dc, w_sb, x_sb, b_sb, dest, ptag, pbufs,
                                half=None, cell={}))

            def queue_proj_v(tg, ptag="pacc", pbufs=1):
                for kx in range(TPG):
                    fillers.append(proj_v_unit(tg, kx, ptag, pbufs))

            def ff_unit(qg, nck, ptag="pacc", pbufs=1, alt_copy=False):
                def unit(qg=qg, nck=nck, ptag=ptag, pbufs=pbufs):
                    fp = ps.tile([P, QGW], F32, tag=ptag, bufs=pbufs,
                                 name=f"fp_{qg}_{nck}")
                    for dc in range(DC_N):
                        nc.tensor.matmul(
                            fp[:],
                            lhsT=wff_sb[:, dc, nck * P:(nck + 1) * P],
                            rhs=at_g[qg][:, dc, :],
                            start=(dc == 0),
                            stop=(dc == DC_N - 1),
                        )
                    ot = opool.tile([P, QGW], BF16, tag="ot",
                                    name=f"ot_{qg}_{nck}")
                    if use_bias:
                        nc.scalar.activation(ot[:], fp[:], AF.Identity,
                                             bias=bffq_sb[:, nck:nck + 1])
                    elif alt_copy and nck % 2:
                        nc.scalar.copy(ot[:], fp[:])
                    else:
                        nc.vector.tensor_copy(ot[:], fp[:])
                    nc.sync.dma_start(
                        outT[nck * P:(nck + 1) * P,
                             qg * QGW:(qg + 1) * QGW], ot[:])
                return unit

            def queue_ff(qg, ptag="pacc", pbufs=1, alt_copy=False):
                for nck in range(KC_N):
                    fillers.append(ff_unit(qg, nck, ptag, pbufs, alt_copy))

            def attention(qg):
                kmax = (qg + 1) * TPG
                op01 = ps.tile([P, QGW], F32, tag="op", bufs=2,
                               name=f"op01_{qg}")
                op23 = ps.tile([P, QGW], F32, tag="op", bufs=2,
                               name=f"op23_{qg}")
                zp = ps.tile([P, QGW], F32, tag="zp", bufs=1,
                             name=f"zp_{qg}")
                budget = max(0, len(fillers) - 3)

                def emit_av_z(kt, ets, off):
                    kg, kx = kt // TPG, kt % TPG
                    for pi, op in enumerate((op01, op23)):
                        et = ets[pi]
                        for j in range(2):  # col-packed AV pair
                            h = pi * 2 + j
                            nc.tensor.matmul(
                                op[j * DH:(j + 1) * DH, off:],
                                lhsT=va_g[kg][:, kx, h * DH:(h + 1) * DH],
                                rhs=et[:, j * QGW + off:(j + 1) * QGW],
                                start=(kt == 0), stop=(kt == kmax - 1),
                            )
                    for h in range(HPC):  # 4-up col-packed Z (M=32)
                        et = ets[h // 2]
                        j = h % 2
                        nc.tensor.matmul(
                            zp[h * 32:(h + 1) * 32, off:],
                            lhsT=onesz[:, :],
                            rhs=et[:, j * QGW + off:(j + 1) * QGW],
                            start=(kt == 0), stop=(kt == kmax - 1),
                            tile_position=(0, h * 32),
                        )

                prev = None
                for kt in range(kmax):
                    quota = (((kt + 1) * budget) // kmax
                             - (kt * budget) // kmax)
                    kg, kx = kt // TPG, kt % TPG
                    off = max(0, kt * P - qg * QGW)
                    sps = []
                    for pi in range(2):  # head pairs (0,1) and (2,3)
                        sp = ps.tile([P, 2 * QGW], F32, tag=f"sp{pi}",
                                     bufs=1, name=f"sp{pi}_{qg}_{kt}")
                        for j in range(2):
                            h = pi * 2 + j
                            po = (h % 2) * DH
                            dch = h // 2
                            nc.tensor.matmul(
                                sp[:, j * QGW + off:(j + 1) * QGW],
                                lhsT=kT_g[kg][po:po + DH, dch,
                                              kx * P:(kx + 1) * P],
                                rhs=qT_g[qg][po:po + DH, dch, off:],
                                start=True, stop=True,
                            )
                        sps.append(sp)
                    ets = []
                    for pi in range(2):
                        sp = sps[pi]
                        et = epool.tile([P, 2 * QGW], BF16, tag="et",
                                        name=f"et{pi}_{qg}_{kt}")
                        # full-width exp even on boundary tiles: the
                        # columns below `off` hold stale scores (bounded)
                        # and are never read downstream
                        nc.scalar.activation(et[:], sp[:], AF.Exp)
                        if kg == qg:  # diagonal block: triangular 0/1 mask
                            for j in range(2):
                                c0 = j * QGW + off
                                nc.vector.tensor_mul(et[:, c0:c0 + P],
                                                     et[:, c0:c0 + P],
                                                     dmask_sb[:])
                        ets.append(et)
                    run_units(fillers, (quota + 1) // 2)
                    if prev is not None:
                        emit_av_z(*prev)
                    prev = (kt, ets, off)
                    run_units(fillers, quota // 2)
                emit_av_z(*prev)
                # ---- normalize: at = op / Z ----
                zr = zpool.tile([P, QGW], F32, tag="zr", name=f"zr_{qg}")
                nc.vector.reciprocal_approx_fast(zr[:], zp[:])
                # move the 4 per-head 1/Z rows to partition 0 (the
                # gpsimd broadcast only reads/writes from partition 0);
                # bf16 + per-head chaining keeps the seam latency short
                zrow = zpool.tile([1, HPC * QGW], BF16, tag="zrow",
                                  name=f"zrow_{qg}")
                ops = (op01, op01, op23, op23)
                for h in range(HPC):
                    lo = (h % 2) * DH
                    if h == 0:
                        # zr row 0 is already at physical partition 0:
                        # broadcast it directly, no staging copy
                        zb = zpool.tile([P, QGW], F32, tag="zb0", bufs=2,
                                        name=f"zb_{qg}_{h}")
                        nc.gpsimd.partition_broadcast(
                            zb[:DH, :], zr[0:1, :], channels=DH)
                    else:
                        nc.vector.tensor_copy(
                            zrow[0:1, h * QGW:(h + 1) * QGW],
                            zr[h * 32:h * 32 + 1, :])
                        zb = zpool.tile([P, QGW], BF16, tag="zb", bufs=6,
                                        name=f"zb_{qg}_{h}")
                        nc.gpsimd.partition_broadcast(
                            zb[:lo + DH, :],
                            zrow[0:1, h * QGW:(h + 1) * QGW],
                            channels=lo + DH)
                    nc.vector.tensor_mul(at_g[qg][lo:lo + DH, h // 2, :],
                                         ops[h][lo:lo + DH, :],
                                         zb[lo:lo + DH, :])

            # ---- schedule ----
            # boot: group-0 projections on the double-buffered "op" banks
            queue_proj_qk(0, ptag="op", pbufs=2, halves=True)
            queue_proj_v(0, ptag="op", pbufs=2)
            run_units(fillers)
            for qg in range(TG_N):
                if qg >= 1:
                    queue_proj_v(qg)   # va[qg] first needed at kt=4*qg
                if qg + 1 < TG_N:
                    queue_proj_qk(qg + 1)
                if qg >= 1:
                    queue_ff(qg - 1)
                attention(qg)
                run_units(fillers)
            queue_ff(TG_N - 1, ptag="op", pbufs=2, alt_copy=True)
            run_units(fillers)

    nc.compile()
    return nc


# ---------------------------------------------------------------------------
# Fallback (dense / generic mask) program: previous-generation implementation.
# ---------------------------------------------------------------------------
def build_program(variant: str, use_bias: bool, s=S, d=D, hpc=HPC,
                  n_devices=NCORES):
    """variant: 'dense' | 'generic'. Returns compiled Bacc."""
    assert variant in ("dense", "generic")
    F32R = mybir.dt.float32r
    kc_n = d // P           # contraction chunks over model dim
    tt = s // P             # token tiles
    hd = hpc * DH           # per-core projected width
    dc_n = hd // P          # dout chunks for QT/KT (and hd chunks for ff)
    tg_n = s // QGW         # token/q groups
    tpg = QGW // P          # token tiles per group (4)
    zw = hpc * QGW          # z columns per qg-pair tile

    nc = bacc.Bacc("TRN2", target_bir_lowering=False, debug=False,
                   num_devices=n_devices)

    def din(name, shape, dt=BF16):
        return nc.dram_tensor(name, list(shape), dt, kind="ExternalInput").ap()

    xqT = din("xqT", (d, s))
    xkT = din("xkT", (d, s))
    xvT = din("xvT", (d, s))
    wqT = din("wqT", (d, hd))   # pre-scaled by 1/sqrt(DH) on host
    wkT = din("wkT", (d, hd))
    wvT = din("wvT", (d, hd))
    wffT = din("wffT", (hd, d))
    if use_bias:
        bq = din("bq", (hd,), F32)   # pre-scaled by 1/sqrt(DH) on host
        bk = din("bk", (hd,), F32)
        bv = din("bv", (1, hd))
        bffq = din("bffq", (d,), F32)    # bff / GPB
        onesb = din("onesb", (1, P))
    if variant == "generic":
        mbT = din("mbT", (s, s), F32)  # mask[b,0].T * -1e9, [k, q] layout
    outT = nc.dram_tensor("outT", [d, s], F32, kind="ExternalOutput").ap()
    zdr = nc.dram_tensor("zdr", [tg_n, hpc * QGW], F32).ap()

    with tile.TileContext(nc) as tc:
        with (
            nc.allow_low_precision(reason="bf16 matmul chain; psum stays fp32"),
            tc.tile_pool(name="consts", bufs=1) as consts,
            tc.tile_pool(name="xin", bufs=1) as xin,
            tc.tile_pool(name="acts", bufs=1) as acts,
            tc.tile_pool(name="epool", bufs=8) as epool,
            tc.tile_pool(name="opool", bufs=4) as opool,
            tc.tile_pool(name="ps", bufs=1, space="PSUM") as ps,
        ):
            # ---- constant / weight loads ----
            wq_sb = consts.tile([P, kc_n, hd], BF16, tag="wq")
            wk_sb = consts.tile([P, kc_n, hd], BF16, tag="wk")
            wv_sb = consts.tile([P, kc_n, hd], BF16, tag="wv")
            wff_sb = consts.tile([P, dc_n, d], BF16, tag="wff")
            nc.sync.dma_start(wq_sb[:], wqT.rearrange("(c p) m -> p c m", p=P))
            _loaded = set()

            def load_w(name, sb, dram):
                if name not in _loaded:
                    _loaded.add(name)
                    nc.sync.dma_start(sb[:],
                                      dram.rearrange("(c p) m -> p c m", p=P))
            if use_bias:
                bq_sb = consts.tile([P, dc_n], F32, tag="bq")
                bk_sb = consts.tile([P, dc_n], F32, tag="bk")
                nc.sync.dma_start(bq_sb[:], bq.rearrange("(c p) -> p c", p=P))
                nc.sync.dma_start(bk_sb[:], bk.rearrange("(c p) -> p c", p=P))
                bv_sb = consts.tile([1, hd], BF16, tag="bv")
                nc.sync.dma_start(bv_sb[:], bv[:])
                bffq_sb = consts.tile([P, kc_n], F32, tag="bffq")
                nc.sync.dma_start(bffq_sb[:],
                                  bffq.rearrange("(c p) -> p c", p=P))
                onesb_sb = consts.tile([1, P], BF16, tag="onesb")
                nc.sync.dma_start(onesb_sb[:], onesb[:])

            # resident bf16 activations for Q/K projections
            xq_sb = acts.tile([P, kc_n, s], BF16, tag="xq")
            xk_sb = acts.tile([P, kc_n, s], BF16, tag="xk")

            qT_g = [acts.tile([P, dc_n, QGW], BF16, tag=f"qT{g}",
                              name=f"qT_{g}") for g in range(tg_n)]
            kT_g = [acts.tile([P, dc_n, QGW], BF16, tag=f"kT{g}",
                              name=f"kT_{g}") for g in range(tg_n)]
            va_g = [acts.tile([P, tpg, hpc * (DH + 1)], BF16, tag=f"va{g}",
                              name=f"va_{g}") for g in range(tg_n)]
            at_g = [acts.tile([P, dc_n, QGW], BF16, tag=f"at{g}",
                              name=f"at_{g}") for g in range(tg_n)]
            z_q = [acts.tile([1, zw], F32, tag=f"z{g % 2}", name=f"z_{g}")
                   for g in range(tg_n)]
            zi_q = [acts.tile([1, zw], F32, tag=f"zi{g % 2}", name=f"zi_{g}")
                    for g in range(tg_n)]
            zb_q = [acts.tile([P, zw], F32, tag=f"zb{g % 2}", name=f"zb_{g}")
                    for g in range(tg_n)]

            _xdma_done = set()

            def load_x(name, x_sb, x_dram):
                if name in _xdma_done:
                    return
                _xdma_done.add(name)
                for kc in range(kc_n):
                    nc.sync.dma_start(x_sb[:, kc, :],
                                      x_dram[kc * P:(kc + 1) * P, :])

            def proj_qk_units(tg, w_sb, x_sb, b_sb, dest, out):
                cell = {}
                for dc in range(dc_n):
                    for half in range(2):
                        def chain(tg=tg, dc=dc, half=half, w_sb=w_sb,
                                  x_sb=x_sb, b_sb=b_sb, dest=dest):
                            if half == 0:
                                cell[dc] = ps.tile([P, QGW], F32, tag="pacc",
                                                   bufs=2,
                                                   name=f"pp_{tg}_{dc}")
                            pp = cell[dc]
                            k0 = half * (kc_n // 2)
                            for kc in range(k0, k0 + kc_n // 2):
                                nc.tensor.matmul(
                                    pp[:],
                                    lhsT=w_sb[:, kc, dc * P:(dc + 1) * P],
                                    rhs=x_sb[:, kc,
                                             tg * QGW:(tg + 1) * QGW],
                                    start=(kc == 0),
                                    stop=(kc == kc_n - 1),
                                )
                            if half == 1:
                                if use_bias:
                                    nc.scalar.activation(
                                        dest[:, dc, :], pp[:], AF.Identity,
                                        bias=b_sb[:, dc:dc + 1])
                                else:
                                    nc.vector.tensor_copy(dest[:, dc, :],
                                                          pp[:])
                        out.append(chain)

            def proj_v_units(tg, out):
                def ones_unit(tg=tg):
                    nc.gpsimd.memset(
                        va_g[tg].rearrange("p t (h e) -> p t h e",
                                           e=DH + 1)[:, :, :, DH], 1.0)
                out.append(ones_unit)
                for ti in range(tpg):
                    def v_unit(tg=tg, ti=ti):
                        t = tg * tpg + ti
                        xvt = xin.tile([P, kc_n, P], BF16, tag="xvstream",
                                       bufs=4, name=f"xvt_{t}")
                        nc.sync.dma_start(
                            xvt[:],
                            xvT[:, t * P:(t + 1) * P].rearrange(
                                "(c p) t -> p c t", p=P))
                        vp = ps.tile([P, QGW], F32, tag="pacc", bufs=2,
                                     name=f"vp_{t}")
                        if use_bias:
                            nc.tensor.matmul(vp[:, :hd],
                                             lhsT=onesb_sb[0:1, :],
                                             rhs=bv_sb[:, :], start=True,
                                             stop=False)
                        for kc in range(kc_n):
                            nc.tensor.matmul(
                                vp[:, :hd],
                                lhsT=xvt[:, kc, :],
                                rhs=wv_sb[:, kc, :],
                                start=(kc == 0 and not use_bias),
                                stop=(kc == kc_n - 1),
                            )
                        nc.vector.tensor_copy(
                            va_g[tg][:, ti].rearrange(
                                "p (h e) -> p h e", e=DH + 1)[:, :, :DH],
                            vp[:, :hd].rearrange("p (h e) -> p h e", e=DH))
                    out.append(v_unit)

            def norm_ff_units(qg, out):
                def mul_unit(qg=qg):
                    for h in range(hpc):
                        dc = (h * DH) // P
                        po = (h * DH) % P
                        nc.vector.tensor_mul(
                            at_g[qg][po:po + DH, dc, :],
                            at_g[qg][po:po + DH, dc, :],
                            zb_q[qg][po:po + DH, h * QGW:(h + 1) * QGW],
                        )
                out.append(mul_unit)
                for nck in range(kc_n):
                    def ff_unit(qg=qg, nck=nck):
                        fp = ps.tile([P, QGW], F32, tag="pacc", bufs=2,
                                     name=f"fp_{nck}_{qg}")
                        for dc in range(dc_n):
                            nc.tensor.matmul(
                                fp[:],
                                lhsT=wff_sb[:, dc, nck * P:(nck + 1) * P],
                                rhs=at_g[qg][:, dc, :],
                                start=(dc == 0),
                                stop=(dc == dc_n - 1),
                            )
                        ot = opool.tile([P, QGW], F32, tag="otile",
                                        name=f"ot_{nck}_{qg}")
                        if use_bias:
                            nc.scalar.activation(ot[:], fp[:], AF.Identity,
                                                 bias=bffq_sb[:, nck:nck + 1])
                        else:
                            nc.vector.tensor_copy(ot[:], fp[:])
                        nc.sync.dma_start(
                            outT[nck * P:(nck + 1) * P,
                                 qg * QGW:(qg + 1) * QGW], ot[:])
                    out.append(ff_unit)

            def run_units(units, n=None):
                k = len(units) if n is None else min(n, len(units))
                for _ in range(k):
                    units.popleft()()

            def attention(qg, fillers):
                kmax = tt
                PW = 2  # score tiles batched per exp
                nquad = kmax // PW
                for h in range(hpc):
                    po = (h * DH) % P
                    dch = (h * DH) // P
                    op = ps.tile([P, QGW], F32, tag="opacc", bufs=2,
                                 name=f"op_{h}_{qg}")
                    ets = [None] * nquad

                    def emit_scores(qd):
                        sp = ps.tile([P, PW * QGW], F32, tag="mmw", bufs=2,
                                     name=f"sp_{h}_{qg}_{qd}")
                        for j in range(PW):
                            kt = qd * PW + j
                            kg, kx = kt // tpg, kt % tpg
                            kh = kT_g[kg][po:po + DH, dch,
                                          kx * P:(kx + 1) * P]
                            nc.tensor.matmul(
                                sp[:, j * QGW:(j + 1) * QGW],
                                lhsT=kh,
                                rhs=qT_g[qg][po:po + DH, dch, :],
                                start=True,
                                stop=True,
                            )
                            if variant == "generic":
                                mb_sb = xin.tile([P, QGW], F32, tag="mstream",
                                                 bufs=4,
                                                 name=f"mb_{h}_{qg}_{kt}")
                                nc.sync.dma_start(
                                    mb_sb[:],
                                    mbT[kt * P:(kt + 1) * P,
                                        qg * QGW:(qg + 1) * QGW])
                                nc.vector.tensor_add(
                                    sp[:, j * QGW:(j + 1) * QGW],
                                    sp[:, j * QGW:(j + 1) * QGW], mb_sb[:])
                        et = epool.tile([P, PW * QGW], BF16, tag="etile",
                                        name=f"et_{h}_{qg}_{qd}")
                        nc.scalar.activation(et[:], sp[:], AF.Exp)
                        ets[qd] = et

                    def emit_av(qd):
                        et = ets[qd]
                        for j in range(PW):
                            kt = qd * PW + j
                            kg, kx = kt // tpg, kt % tpg
                            nc.tensor.matmul(
                                op[:DH + 1, :],
                                lhsT=va_g[kg][:, kx, h * (DH + 1):
                                              (h + 1) * (DH + 1)],
                                rhs=et[:, j * QGW:(j + 1) * QGW],
                                start=(kt == 0),
                                stop=(kt == kmax - 1),
                            )
                        ets[qd] = None

                    emit_scores(0)
                    for qd in range(1, nquad):
                        emit_scores(qd)
                        run_units(fillers, 1)
                        emit_av(qd - 1)
                    emit_av(nquad - 1)
                    run_units(fillers, 1)
                    nc.vector.tensor_copy(
                        at_g[qg][po:po + DH, dch, :], op[:DH, :])
                    nc.vector.tensor_copy(
                        z_q[qg][0:1, h * QGW:(h + 1) * QGW],
                        op[DH:DH + 1, :])
                    hs = slice(h * QGW, (h + 1) * QGW)
                    nc.vector.reciprocal_approx_fast(zi_q[qg][0:1, hs],
                                                     z_q[qg][0:1, hs])
                    nc.sync.dma_start(zdr[qg:qg + 1, hs], zi_q[qg][0:1, hs])
                    nc.sync.dma_start(
                        zb_q[qg][:, hs],
                        zdr[qg:qg + 1, hs].to_broadcast([P, QGW]))

            from collections import deque
            fillers = deque()

            def queue_proj(tg):
                fillers.append(lambda: load_x("xq", xq_sb, xqT))
                proj_qk_units(tg, wq_sb, xq_sb, bq_sb if use_bias else None,
                              qT_g[tg], fillers)
                fillers.append(lambda: load_w("wk", wk_sb, wkT))
                fillers.append(lambda: load_x("xk", xk_sb, xkT))
                proj_qk_units(tg, wk_sb, xk_sb, bk_sb if use_bias else None,
                              kT_g[tg], fillers)
                fillers.append(lambda: load_w("wv", wv_sb, wvT))
                proj_v_units(tg, fillers)

            for tg in range(tg_n):
                queue_proj(tg)
                run_units(fillers)
            load_w("wff", wff_sb, wffT)
            for qg in range(tg_n):
                if qg > 0:
                    norm_ff_units(qg - 1, fillers)
                attention(qg, fillers)
                run_units(fillers)
            norm_ff_units(tg_n - 1, fillers)
            run_units(fillers)

    nc.compile()
    return nc


def _classify_mask(mask: np.ndarray) -> str:
    m = np.asarray(mask)[:, 0]  # [B, S, S]
    if not m.any():
        return "dense"
    s = m.shape[-1]
    causal = np.triu(np.ones((s, s), dtype=m.dtype), k=1)
    if all(np.array_equal(m[b], causal) for b in range(m.shape[0])):
        return "causal"
    return "generic"


def _bf(x):
    return np.ascontiguousarray(np.ascontiguousarray(x).astype(NPBF16))


def _make_in_maps(variant, query, key, value, mask, wq, bq, wk, bk, wv, bv,
                  wff, bff, use_bias):
    scale = np.float32(1.0 / np.sqrt(np.float32(DH)))
    if variant == "causal":
        # prepacked group-major layouts (all transforms on host, free)
        wqs = (wq * scale).T.reshape(KC_N, P, D)   # [kc, p, m_full]
        wkT = wk.T.reshape(KC_N, P, D)
        wvT = wv.T.reshape(KC_N, P, D)
        wffT = wff.T                                # [d_in=1024? no: (D, D)]

        def xpack(x, b):
            # x[b].T [D, S] -> [tg, p, kc, j]
            xt = _bf(x[b].T)
            return np.ascontiguousarray(
                xt.reshape(KC_N, P, TG_N, QGW).transpose(2, 1, 0, 3))

        xq_p = [xpack(query, b) for b in range(B)]
        xk_p = [xpack(key, b) for b in range(B)]
        xv_p = [xpack(value, b) for b in range(B)]
        dmask = np.tril(np.ones((P, P), np.float32)).T  # [k,q] 1 if k<=q

        in_maps = []
        for c in range(NCORES):
            b, hg = c // GPB, c % GPB
            sl = slice(hg * HD, (hg + 1) * HD)
            m = {
                "xq": xq_p[b], "xk": xk_p[b], "xv": xv_p[b],
                "wq": _bf(wqs[:, :, sl].transpose(1, 0, 2)),
                "wk": _bf(wkT[:, :, sl].transpose(1, 0, 2)),
                "wv": _bf(wvT[:, :, sl].transpose(1, 0, 2)),
                # wff rows for this head slice: [256, 1024] -> [p, dc, n]
                "wff": _bf(wff.T[sl, :].reshape(DC_N, P, D)
                           .transpose(1, 0, 2)),
                "dmask": _bf(dmask),
            }
            if use_bias:
                m["bq"] = np.ascontiguousarray(
                    (bq * scale)[sl].reshape(DC_N, P).T).astype(np.float32)
                m["bk"] = np.ascontiguousarray(
                    bk[sl].reshape(DC_N, P).T).astype(np.float32)
                m["bv"] = _bf(bv[sl])[None, :]
                m["bffq"] = np.ascontiguousarray(
                    (bff / GPB).reshape(KC_N, P).T).astype(np.float32)
                m["onesb"] = np.ones((1, P), NPBF16)
            in_maps.append(m)
        return in_maps

    # fallback variants (dense / generic)
    wqTs = _bf((wq * scale).T)
    wkT = _bf(wk.T)
    wvT = _bf(wv.T)
    wffT = _bf(wff.T)

    qT = [_bf(query[b].T) for b in range(B)]
    kT = [_bf(key[b].T) for b in range(B)]
    vT = [_bf(value[b].T) for b in range(B)]
    mbT = None
    if variant == "generic":
        mbT = [np.ascontiguousarray(mask[b, 0].T * np.float32(-1e9))
               for b in range(B)]

    in_maps = []
    for c in range(NCORES):
        b, hg = c // GPB, c % GPB
        sl = slice(hg * HD, (hg + 1) * HD)
        m = {
            "xqT": qT[b], "xkT": kT[b], "xvT": vT[b],
            "wqT": np.ascontiguousarray(wqTs[:, sl]),
            "wkT": np.ascontiguousarray(wkT[:, sl]),
            "wvT": np.ascontiguousarray(wvT[:, sl]),
            "wffT": np.ascontiguousarray(wffT[sl, :]),
        }
        if use_bias:
            m["bq"] = np.ascontiguousarray((bq * scale)[sl]).astype(np.float32)
            m["bk"] = np.ascontiguousarray(bk[sl]).astype(np.float32)
            m["bv"] = _bf(bv[sl])[None, :]
            m["bffq"] = (bff / GPB).astype(np.float32)
            m["onesb"] = np.ones((1, P), NPBF16)
        if variant == "generic":
            m["mbT"] = mbT[b]
        in_maps.append(m)
    return in_maps


def kernel(**inputs) -> np.ndarray:
    query = np.ascontiguousarray(inputs["query"], dtype=np.float32)
    key = np.ascontiguousarray(inputs["key"], dtype=np.float32)
    value = np.ascontiguousarray(inputs["value"], dtype=np.float32)
    mask = np.asarray(inputs["mask"], dtype=np.float32)
    wq = np.asarray(inputs["wq"], np.float32)
    bq = np.asarray(inputs["bq"], np.float32)
    wk = np.asarray(inputs["wk"], np.float32)
    bk = np.asarray(inputs["bk"], np.float32)
    wv = np.asarray(inputs["wv"], np.float32)
    bv = np.asarray(inputs["bv"], np.float32)
    wff = np.asarray(inputs["wff"], np.float32)
    bff = np.asarray(inputs["bff"], np.float32)

    variant = _classify_mask(mask)
    use_bias = bool(bq.any() or bk.any() or bv.any() or bff.any())
    pkey = (variant, use_bias)
    if pkey not in _PROG_CACHE:
        if variant == "causal":
            _PROG_CACHE[pkey] = build_causal(use_bias)
        else:
            _PROG_CACHE[pkey] = build_program(variant, use_bias)
    nc = _PROG_CACHE[pkey]

    in_maps = _make_in_maps(variant, query, key, value, mask, wq, bq, wk, bk,
                            wv, bv, wff, bff, use_bias)
    res = run_bass_kernel_spmd(nc, in_maps, core_ids=list(range(NCORES)))
    out = np.empty((B, S, D), np.float32)
    for b in range(B):
        acc = res.results[b * GPB]["outT"].astype(np.float32)
        for g in range(1, GPB):
            acc = acc + res.results[b * GPB + g]["outT"].astype(np.float32)
        out[b] = acc.T
    return out


if __name__ == "__main__":
    import reference

    inputs = {k: np.asarray(v) for k, v in reference.setup_inputs().items()}
    out = kernel(**inputs)
    print("kernel out:", out.shape, out.dtype)


# revision 26
# speedup vs baseline: 1.0127x; 1.0127x over previous
"""Trainium2 Bass kernel for nn_MultiHeadAttention (B=2, S=2048, D=1024, H=16).

Sharding: 8 cores = 2 batches x 4 head-groups (4 heads per core, tensor
parallel over heads). Each core computes, for its batch b and its 4 heads:
  QT/KT = (x @ W.T).T projections in transposed layout [256, 2048]
  V     = value @ wv.T in normal [token, dh] layout
  E^T   = exp(scoresT) tiles [k, q] straight from matmul (scores are O(1)
          here so exp without max-subtraction is safe; causal masking via a
          multiplicative 0/1 tile on the diagonal blocks)
  out   = (V^T E^T) / Z  with Z = colsum(E^T) from a ones-lhsT matmul
  ffT   = wff_rows-partial @ at -> [1024, 2048] bf16 partial, host-summed.

PE-array packing (tile_position derived from base partitions):
  scores: contraction is DH=64, so heads (h0,h1)/(h2,h3) run ROW-packed
          (two concurrent matmuls in array rows 0-63 / 64-127).
  AV:     output M=64 per head, so head pairs run COL-packed
          (array cols 0-63 / 64-127 -> psum partitions 0-63 / 64-127).
  Z:      4 concurrent M=32 matmuls (ones lhsT) at col positions
          0/32/64/96 -> zp psum rows fully covered, one reciprocal per
          q-group, gpsimd partition_broadcast for the per-head 1/Z rows.
A warm-up spinner issues dummy matmuls at t=0 so the PE HAM clock-gate
reaches 2.4 GHz before real work lands; inputs are host-prepacked into
group-major contiguous blocks so the first projection starts ~4us in.
"""

import sys

sys.path.insert(0, "/opt/trn_rl_repo")

import ml_dtypes
import numpy as np

import concourse.bass as bass
import concourse.mybir as mybir
import concourse.tile as tile
from concourse import bacc
from concourse import library_config
from concourse.bass_utils import run_bass_kernel_spmd

P = 128
B, S, D, H = 2, 2048, 1024, 16
DH = D // H  # 64
NCORES = 8
GPB = NCORES // B  # cores (head groups) per batch = 4
HPC = H // GPB  # heads per core = 4
HD = HPC * DH  # projected cols per core = 256
F32 = mybir.dt.float32
BF16 = mybir.dt.bfloat16
QGW = 512  # q-group width (psum free dim)
AF = mybir.ActivationFunctionType
NPBF16 = ml_dtypes.bfloat16

KC_N = D // P  # 8 contraction chunks over model dim
DC_N = HD // P  # 2 dout chunks
TG_N = S // QGW  # 4 token groups
TPG = QGW // P  # 4 token tiles per group

USE_GPSIMD_BCAST = True
NSPIN = 54

_PROG_CACHE: dict = {}


def build_causal(use_bias: bool):
    """Optimized causal-mask program with PE tile packing."""
    nc = bacc.Bacc("TRN2", target_bir_lowering=False, debug=False,
                   num_devices=NCORES)

    def din(name, shape, dt=BF16):
        return nc.dram_tensor(name, list(shape), dt, kind="ExternalInput").ap()

    xq_d = din("xq", (TG_N, P, KC_N, QGW))
    xk_d = din("xk", (TG_N, P, KC_N, QGW))
    xv_d = din("xv", (TG_N, P, KC_N, QGW))
    wq_d = din("wq", (P, KC_N, HD))   # pre-scaled by 1/sqrt(DH) on host
    wk_d = din("wk", (P, KC_N, HD))
    wv_d = din("wv", (P, KC_N, HD))
    wff_d = din("wff", (P, DC_N, D))
    dmask_d = din("dmask", (P, P))    # [k, q]: 1 if k <= q else 0
    if use_bias:
        bq_d = din("bq", (P, DC_N), F32)   # pre-scaled by 1/sqrt(DH)
        bk_d = din("bk", (P, DC_N), F32)
        bv_d = din("bv", (1, HD))
        bffq_d = din("bffq", (P, KC_N), F32)  # bff / GPB
        onesb_d = din("onesb", (1, P))
    outT = nc.dram_tensor("outT", [D, S], BF16, kind="ExternalOutput").ap()
    zdr = nc.dram_tensor("zdr", [TG_N * HPC, QGW], BF16).ap()

    with tile.TileContext(nc) as tc:
        with (
            nc.allow_low_precision(reason="bf16 matmul chain; psum stays f32"),
            tc.tile_pool(name="consts", bufs=1) as consts,
            tc.tile_pool(name="acts", bufs=1) as acts,
            tc.tile_pool(name="zpool", bufs=2) as zpool,
            tc.tile_pool(name="epool", bufs=6) as epool,
            tc.tile_pool(name="opool", bufs=4) as opool,
            tc.tile_pool(name="ps", bufs=1, space="PSUM") as ps,
        ):
            if USE_GPSIMD_BCAST:
                nc.gpsimd.load_library(library_config.attn)

            # ---- warm-up spinner (keeps PE busy so HAM un-throttles) ----
            spin = consts.tile([P, QGW], BF16, tag="spin")
            nc.vector.memset(spin[:], 0.0)
            onesz = consts.tile([P, 32], BF16, tag="onesz")
            nc.vector.memset(onesz[:], 1.0)
            spin_ps = ps.tile([P, QGW], F32, tag="op", bufs=2, name="spin_ps")
            for i in range(NSPIN):
                nc.tensor.matmul(spin_ps[:, :256], lhsT=spin[:, :P],
                                 rhs=spin[:, :256], start=True, stop=True)
            junk = consts.tile([P, 1], F32, tag="junk")
            nc.vector.tensor_copy(junk[:], spin_ps[:, 0:1])

            # ---- constant / weight / activation tiles ----
            dmask_sb = consts.tile([P, P], BF16, tag="dmask")
            wq_sb = consts.tile([P, KC_N, HD], BF16, tag="wq")
            wk_sb = consts.tile([P, KC_N, HD], BF16, tag="wk")
            wv_sb = consts.tile([P, KC_N, HD], BF16, tag="wv")
            wff_sb = consts.tile([P, DC_N, D], BF16, tag="wff")
            xq_g = [acts.tile([P, KC_N, QGW], BF16, tag=f"xq{g}")
                    for g in range(TG_N)]
            xk_g = [acts.tile([P, KC_N, QGW], BF16, tag=f"xk{g}")
                    for g in range(TG_N)]
            xv_g = [acts.tile([P, KC_N, QGW], BF16, tag=f"xv{g}")
                    for g in range(TG_N)]
            if use_bias:
                bq_sb = consts.tile([P, DC_N], F32, tag="bq")
                bk_sb = consts.tile([P, DC_N], F32, tag="bk")
                bv_sb = consts.tile([1, HD], BF16, tag="bv")
                bffq_sb = consts.tile([P, KC_N], F32, tag="bffq")
                onesb_sb = consts.tile([1, P], BF16, tag="onesb")

            # DMA emission order = start priority: group-0 activations
            # first, kc-halves separate so half-chains start early.
            nc.sync.dma_start(wq_sb[:], wq_d[:])
            nc.sync.dma_start(xq_g[0][:, 0:4, :], xq_d[0, :, 0:4, :])
            nc.sync.dma_start(wk_sb[:], wk_d[:])
            nc.sync.dma_start(xk_g[0][:, 0:4, :], xk_d[0, :, 0:4, :])
            nc.sync.dma_start(xq_g[0][:, 4:8, :], xq_d[0, :, 4:8, :])
            nc.sync.dma_start(xk_g[0][:, 4:8, :], xk_d[0, :, 4:8, :])
            nc.sync.dma_start(wv_sb[:], wv_d[:])
            nc.sync.dma_start(xv_g[0][:], xv_d[0])
            nc.sync.dma_start(dmask_sb[:], dmask_d[:])
            if use_bias:
                nc.sync.dma_start(bq_sb[:], bq_d[:])
                nc.sync.dma_start(bk_sb[:], bk_d[:])
                nc.sync.dma_start(bv_sb[:], bv_d[:])
                nc.sync.dma_start(bffq_sb[:], bffq_d[:])
                nc.sync.dma_start(onesb_sb[:], onesb_d[:])
            for g in (1, 2, 3):
                nc.sync.dma_start(xq_g[g][:], xq_d[g])
                nc.sync.dma_start(xk_g[g][:], xk_d[g])
                nc.sync.dma_start(xv_g[g][:], xv_d[g])
                if g == 1:
                    nc.sync.dma_start(wff_sb[:], wff_d[:])

            # per-group projected activations
            qT_g = [acts.tile([P, DC_N, QGW], BF16, tag=f"qT{g}")
                    for g in range(TG_N)]
            kT_g = [acts.tile([P, DC_N, QGW], BF16, tag=f"kT{g}")
                    for g in range(TG_N)]
            va_g = [acts.tile([P, TPG, HD], BF16, tag=f"va{g}")
                    for g in range(TG_N)]
            at_g = [acts.tile([P, DC_N, QGW], BF16, tag=f"at{g}")
                    for g in range(TG_N)]

            from collections import deque
            fillers = deque()

            def run_units(units, n=None):
                k = len(units) if n is None else min(n, len(units))
                for _ in range(k):
                    units.popleft()()

            def proj_qk_unit(tg, dc, w_sb, x_sb, b_sb, dest, ptag, pbufs,
                             half=None, cell=None):
                def unit(tg=tg, dc=dc, w_sb=w_sb, x_sb=x_sb, b_sb=b_sb,
                         dest=dest, ptag=ptag, pbufs=pbufs):
                    if half in (None, 0):
                        cell[0] = ps.tile([P, QGW], F32, tag=ptag, bufs=pbufs,
                                          name=f"pqk_{id(w_sb)}_{tg}_{dc}")
                    pp = cell[0]
                    k0 = 0 if half in (None, 0) else KC_N // 2
                    k1 = KC_N // 2 if half == 0 else KC_N
                    for kc in range(k0, k1):
                        nc.tensor.matmul(
                            pp[:],
                            lhsT=w_sb[:, kc, dc * P:(dc + 1) * P],
                            rhs=x_sb[:, kc, :],
                            start=(kc == 0),
                            stop=(kc == KC_N - 1),
                        )
                    if half == 0:
                        return
                    if use_bias:
                        nc.scalar.activation(dest[:, dc, :], pp[:],
                                             AF.Identity,
                                             bias=b_sb[:, dc:dc + 1])
                    else:
                        nc.vector.tensor_copy(dest[:, dc, :], pp[:])
                return unit

            def proj_v_unit(tg, kx, ptag, pbufs):
                def unit(tg=tg, kx=kx, ptag=ptag, pbufs=pbufs):
                    vp = ps.tile([P, QGW], F32, tag=ptag, bufs=pbufs,
                                 name=f"vp_{tg}_{kx}")
                    if use_bias:
                        nc.tensor.matmul(vp[:, :HD], lhsT=onesb_sb[0:1, :],
                                         rhs=bv_sb[:, :], start=True,
                                         stop=False)
                    for kc in range(KC_N):
                        nc.tensor.matmul(
                            vp[:, :HD],
                            lhsT=xv_g[tg][:, kc, kx * P:(kx + 1) * P],
                            rhs=wv_sb[:, kc, :],
                            start=(kc == 0 and not use_bias),
                            stop=(kc == KC_N - 1),
                        )
                    nc.vector.tensor_copy(va_g[tg][:, kx, :], vp[:, :HD])
                return unit

            def queue_proj_qk(tg, ptag="pacc", pbufs=1, halves=False):
                for w_sb, x_sb, b_sb, dest in (
                        (wq_sb, xq_g[tg], bq_sb if use_bias else None,
                         qT_g[tg]),
                        (wk_sb, xk_g[tg], bk_sb if use_bias else None,
                         kT_g[tg])):
                    for dc in range(DC_N):
                        if halves:
                            cell = {}
                            for half in range(2):
                                fillers.append(proj_qk_unit(
                                    tg, dc, w_sb, x_sb, b_sb, dest, ptag,
                                    pbufs, half=half, cell=cell))
                        else:
                            fillers.append(proj_qk_unit(
                                tg, dc, w_sb, x_sb, b_sb, dest, ptag, pbufs,
                                half=None, cell={}))

            def queue_proj_v(tg, ptag="pacc", pbufs=1):
                for kx in range(TPG):
                    fillers.append(proj_v_unit(tg, kx, ptag, pbufs))

            def ff_unit(qg, nck, ptag="pacc", pbufs=1, alt_copy=False):
                def unit(qg=qg, nck=nck, ptag=ptag, pbufs=pbufs):
                    fp = ps.tile([P, QGW], F32, tag=ptag, bufs=pbufs,
                                 name=f"fp_{qg}_{nck}")
                    for dc in range(DC_N):
                        nc.tensor.matmul(
                            fp[:],
                            lhsT=wff_sb[:, dc, nck * P:(nck + 1) * P],
                            rhs=at_g[qg][:, dc, :],
                            start=(dc == 0),
                            stop=(dc == DC_N - 1),
                        )
                    ot = opool.tile([P, QGW], BF16, tag="ot",
                                    name=f"ot_{qg}_{nck}")
                    if use_bias:
                        nc.scalar.activation(ot[:], fp[:], AF.Identity,
                                             bias=bffq_sb[:, nck:nck + 1])
                    elif alt_copy and nck % 2:
                        nc.scalar.copy(ot[:], fp[:])
                    else:
                        nc.vector.tensor_copy(ot[:], fp[:])
                    nc.sync.dma_start(
                        outT[nck * P:(nck + 1) * P,
                             qg * QGW:(qg + 1) * QGW], ot[:])
                return unit

            def queue_ff(qg, ptag="pacc", pbufs=1, alt_copy=False):
                for nck in range(KC_N):
                    fillers.append(ff_unit(qg, nck, ptag, pbufs, alt_copy))

            def attention(qg):
                kmax = (qg + 1) * TPG
                op01 = ps.tile([P, QGW], F32, tag="op", bufs=2,
                               name=f"op01_{qg}")
                op23 = ps.tile([P, QGW], F32, tag="op", bufs=2,
                               name=f"op23_{qg}")
                zp = ps.tile([P, QGW], F32, tag="zp", bufs=1,
                             name=f"zp_{qg}")
                budget = len(fillers)

                def emit_av_z(kt, ets, off):
                    kg, kx = kt // TPG, kt % TPG
                    for pi, op in enumerate((op01, op23)):
                        et = ets[pi]
                        for j in range(2):  # col-packed AV pair
                            h = pi * 2 + j
                            nc.tensor.matmul(
                                op[j * DH:(j + 1) * DH, off:],
                                lhsT=va_g[kg][:, kx, h * DH:(h + 1) * DH],
                                rhs=et[:, j * QGW + off:(j + 1) * QGW],
                                start=(kt == 0), stop=(kt == kmax - 1),
                            )
                    for h in range(HPC):  # 4-up col-packed Z (M=32)
                        et = ets[h // 2]
                        j = h % 2
                        nc.tensor.matmul(
                            zp[h * 32:(h + 1) * 32, off:],
                            lhsT=onesz[:, :],
                            rhs=et[:, j * QGW + off:(j + 1) * QGW],
                            start=(kt == 0), stop=(kt == kmax - 1),
                            tile_position=(0, h * 32),
                        )

                prev = None
                for kt in range(kmax):
                    quota = (((kt + 1) * budget) // kmax
                             - (kt * budget) // kmax)
                    kg, kx = kt // TPG, kt % TPG
                    off = max(0, kt * P - qg * QGW)
                    sps = []
                    for pi in range(2):  # head pairs (0,1) and (2,3)
                        sp = ps.tile([P, 2 * QGW], F32, tag=f"sp{pi}",
                                     bufs=1, name=f"sp{pi}_{qg}_{kt}")
                        for j in range(2):
                            h = pi * 2 + j
                            po = (h % 2) * DH
                            dch = h // 2
                            nc.tensor.matmul(
                                sp[:, j * QGW + off:(j + 1) * QGW],
                                lhsT=kT_g[kg][po:po + DH, dch,
                                              kx * P:(kx + 1) * P],
                                rhs=qT_g[qg][po:po + DH, dch, off:],
                                start=True, stop=True,
                            )
                        sps.append(sp)
                    ets = []
                    for pi in range(2):
                        sp = sps[pi]
                        et = epool.tile([P, 2 * QGW], BF16, tag="et",
                                        name=f"et{pi}_{qg}_{kt}")
                        # full-width exp even on boundary tiles: the
                        # columns below `off` hold stale scores (bounded)
                        # and are never read downstream
                        nc.scalar.activation(et[:], sp[:], AF.Exp)
                        if kg == qg:  # diagonal block: triangular 0/1 mask
                            for j in range(2):
                                c0 = j * QGW + off
                                nc.vector.tensor_mul(et[:, c0:c0 + P],
                                                     et[:, c0:c0 + P],
                                                     dmask_sb[:])
                        ets.append(et)
                    run_units(fillers, (quota + 1) // 2)
                    if prev is not None:
                        emit_av_z(*prev)
                    prev = (kt, ets, off)
                    run_units(fillers, quota // 2)
                emit_av_z(*prev)
                # ---- normalize: at = op / Z ----
                zr = zpool.tile([P, QGW], F32, tag="zr", name=f"zr_{qg}")
                nc.vector.reciprocal_approx_fast(zr[:], zp[:])
                # move the 4 per-head 1/Z rows to partition 0 (the
                # gpsimd broadcast only reads/writes from partition 0);
                # bf16 + per-head chaining keeps the seam latency short
                zrow = zpool.tile([1, HPC * QGW], BF16, tag="zrow",
                                  name=f"zrow_{qg}")
                ops = (op01, op01, op23, op23)
                for h in range(HPC):
                    nc.vector.tensor_copy(zrow[0:1, h * QGW:(h + 1) * QGW],
                                          zr[h * 32:h * 32 + 1, :])
                    zb = zpool.tile([P, QGW], BF16, tag="zb", bufs=6,
                                    name=f"zb_{qg}_{h}")
                    lo = (h % 2) * DH
                    nc.gpsimd.partition_broadcast(
                        zb[:lo + DH, :], zrow[0:1, h * QGW:(h + 1) * QGW],
                        channels=lo + DH)
                    nc.vector.tensor_mul(at_g[qg][lo:lo + DH, h // 2, :],
                                         ops[h][lo:lo + DH, :],
                                         zb[lo:lo + DH, :])

            # ---- schedule ----
            # boot: group-0 projections on the double-buffered "op" banks
            queue_proj_qk(0, ptag="op", pbufs=2, halves=True)
            queue_proj_v(0, ptag="op", pbufs=2)
            run_units(fillers)
            for qg in range(TG_N):
                if qg >= 1:
                    queue_proj_v(qg)   # va[qg] first needed at kt=4*qg
                if qg + 1 < TG_N:
                    queue_proj_qk(qg + 1)
                if qg >= 1:
                    queue_ff(qg - 1)
                attention(qg)
                run_units(fillers)
            queue_ff(TG_N - 1, ptag="op", pbufs=2, alt_copy=True)
            run_units(fillers)

    nc.compile()
    return nc


# ---------------------------------------------------------------------------
# Fallback (dense / generic mask) program: previous-generation implementation.
# ---------------------------------------------------------------------------
def build_program(variant: str, use_bias: bool, s=S, d=D, hpc=HPC,
                  n_devices=NCORES):
    """variant: 'dense' | 'generic'. Returns compiled Bacc."""
    assert variant in ("dense", "generic")
    F32R = mybir.dt.float32r
    kc_n = d // P           # contraction chunks over model dim
    tt = s // P             # token tiles
    hd = hpc * DH           # per-core projected width
    dc_n = hd // P          # dout chunks for QT/KT (and hd chunks for ff)
    tg_n = s // QGW         # token/q groups
    tpg = QGW // P          # token tiles per group (4)
    zw = hpc * QGW          # z columns per qg-pair tile

    nc = bacc.Bacc("TRN2", target_bir_lowering=False, debug=False,
                   num_devices=n_devices)

    def din(name, shape, dt=BF16):
        return nc.dram_tensor(name, list(shape), dt, kind="ExternalInput").ap()

    xqT = din("xqT", (d, s))
    xkT = din("xkT", (d, s))
    xvT = din("xvT", (d, s))
    wqT = din("wqT", (d, hd))   # pre-scaled by 1/sqrt(DH) on host
    wkT = din("wkT", (d, hd))
    wvT = din("wvT", (d, hd))
    wffT = din("wffT", (hd, d))
    if use_bias:
        bq = din("bq", (hd,), F32)   # pre-scaled by 1/sqrt(DH) on host
        bk = din("bk", (hd,), F32)
        bv = din("bv", (1, hd))
        bffq = din("bffq", (d,), F32)    # bff / GPB
        onesb = din("onesb", (1, P))
    if variant == "generic":
        mbT = din("mbT", (s, s), F32)  # mask[b,0].T * -1e9, [k, q] layout
    outT = nc.dram_tensor("outT", [d, s], F32, kind="ExternalOutput").ap()
    zdr = nc.dram_tensor("zdr", [tg_n, hpc * QGW], F32).ap()

    with tile.TileContext(nc) as tc:
        with (
            nc.allow_low_precision(reason="bf16 matmul chain; psum stays fp32"),
            tc.tile_pool(name="consts", bufs=1) as consts,
            tc.tile_pool(name="xin", bufs=1) as xin,
            tc.tile_pool(name="acts", bufs=1) as acts,
            tc.tile_pool(name="epool", bufs=8) as epool,
            tc.tile_pool(name="opool", bufs=4) as opool,
            tc.tile_pool(name="ps", bufs=1, space="PSUM") as ps,
        ):
            # ---- constant / weight loads ----
            wq_sb = consts.tile([P, kc_n, hd], BF16, tag="wq")
            wk_sb = consts.tile([P, kc_n, hd], BF16, tag="wk")
            wv_sb = consts.tile([P, kc_n, hd], BF16, tag="wv")
            wff_sb = consts.tile([P, dc_n, d], BF16, tag="wff")
            nc.sync.dma_start(wq_sb[:], wqT.rearrange("(c p) m -> p c m", p=P))
            _loaded = set()

            def load_w(name, sb, dram):
                if name not in _loaded:
                    _loaded.add(name)
                    nc.sync.dma_start(sb[:],
                                      dram.rearrange("(c p) m -> p c m", p=P))
            if use_bias:
                bq_sb = consts.tile([P, dc_n], F32, tag="bq")
                bk_sb = consts.tile([P, dc_n], F32, tag="bk")
                nc.sync.dma_start(bq_sb[:], bq.rearrange("(c p) -> p c", p=P))
                nc.sync.dma_start(bk_sb[:], bk.rearrange("(c p) -> p c", p=P))
                bv_sb = consts.tile([1, hd], BF16, tag="bv")
                nc.sync.dma_start(bv_sb[:], bv[:])
                bffq_sb = consts.tile([P, kc_n], F32, tag="bffq")
                nc.sync.dma_start(bffq_sb[:],
                                  bffq.rearrange("(c p) -> p c", p=P))
                onesb_sb = consts.tile([1, P], BF16, tag="onesb")
                nc.sync.dma_start(onesb_sb[:], onesb[:])

            # resident bf16 activations for Q/K projections
            xq_sb = acts.tile([P, kc_n, s], BF16, tag="xq")
            xk_sb = acts.tile([P, kc_n, s], BF16, tag="xk")

            qT_g = [acts.tile([P, dc_n, QGW], BF16, tag=f"qT{g}",
                              name=f"qT_{g}") for g in range(tg_n)]
            kT_g = [acts.tile([P, dc_n, QGW], BF16, tag=f"kT{g}",
                              name=f"kT_{g}") for g in range(tg_n)]
            va_g = [acts.tile([P, tpg, hpc * (DH + 1)], BF16, tag=f"va{g}",
                              name=f"va_{g}") for g in range(tg_n)]
            at_g = [acts.tile([P, dc_n, QGW], BF16, tag=f"at{g}",
                              name=f"at_{g}") for g in range(tg_n)]
            z_q = [acts.tile([1, zw], F32, tag=f"z{g % 2}", name=f"z_{g}")
                   for g in range(tg_n)]
            zi_q = [acts.tile([1, zw], F32, tag=f"zi{g % 2}", name=f"zi_{g}")
                    for g in range(tg_n)]
            zb_q = [acts.tile([P, zw], F32, tag=f"zb{g % 2}", name=f"zb_{g}")
                    for g in range(tg_n)]

            _xdma_done = set()

            def load_x(name, x_sb, x_dram):
                if name in _xdma_done:
                    return
                _xdma_done.add(name)
                for kc in range(kc_n):
                    nc.sync.dma_start(x_sb[:, kc, :],
                                      x_dram[kc * P:(kc + 1) * P, :])

            def proj_qk_units(tg, w_sb, x_sb, b_sb, dest, out):
                cell = {}
                for dc in range(dc_n):
                    for half in range(2):
                        def chain(tg=tg, dc=dc, half=half, w_sb=w_sb,
                                  x_sb=x_sb, b_sb=b_sb, dest=dest):
                            if half == 0:
                                cell[dc] = ps.tile([P, QGW], F32, tag="pacc",
                                                   bufs=2,
                                                   name=f"pp_{tg}_{dc}")
                            pp = cell[dc]
                            k0 = half * (kc_n // 2)
                            for kc in range(k0, k0 + kc_n // 2):
                                nc.tensor.matmul(
                                    pp[:],
                                    lhsT=w_sb[:, kc, dc * P:(dc + 1) * P],
                                    rhs=x_sb[:, kc,
                                             tg * QGW:(tg + 1) * QGW],
                                    start=(kc == 0),
                                    stop=(kc == kc_n - 1),
                                )
                            if half == 1:
                                if use_bias:
                                    nc.scalar.activation(
                                        dest[:, dc, :], pp[:], AF.Identity,
                                        bias=b_sb[:, dc:dc + 1])
                                else:
                                    nc.vector.tensor_copy(dest[:, dc, :],
                                                          pp[:])
                        out.append(chain)

            def proj_v_units(tg, out):
                def ones_unit(tg=tg):
                    nc.gpsimd.memset(
                        va_g[tg].rearrange("p t (h e) -> p t h e",
                                           e=DH + 1)[:, :, :, DH], 1.0)
                out.append(ones_unit)
                for ti in range(tpg):
                    def v_unit(tg=tg, ti=ti):
                        t = tg * tpg + ti
                        xvt = xin.tile([P, kc_n, P], BF16, tag="xvstream",
                                       bufs=4, name=f"xvt_{t}")
                        nc.sync.dma_start(
                            xvt[:],
                            xvT[:, t * P:(t + 1) * P].rearrange(
                                "(c p) t -> p c t", p=P))
                        vp = ps.tile([P, QGW], F32, tag="pacc", bufs=2,
                                     name=f"vp_{t}")
                        if use_bias:
                            nc.tensor.matmul(vp[:, :hd],
                                             lhsT=onesb_sb[0:1, :],
                                             rhs=bv_sb[:, :], start=True,
                                             stop=False)
                        for kc in range(kc_n):
                            nc.tensor.matmul(
                                vp[:, :hd],
                                lhsT=xvt[:, kc, :],
                                rhs=wv_sb[:, kc, :],
                                start=(kc == 0 and not use_bias),
                                stop=(kc == kc_n - 1),
                            )
                        nc.vector.tensor_copy(
                            va_g[tg][:, ti].rearrange(
                                "p (h e) -> p h e", e=DH + 1)[:, :, :DH],
                            vp[:, :hd].rearrange("p (h e) -> p h e", e=DH))
                    out.append(v_unit)

            def norm_ff_units(qg, out):
                def mul_unit(qg=qg):
                    for h in range(hpc):
                        dc = (h * DH) // P
                        po = (h * DH) % P
                        nc.vector.tensor_mul(
                            at_g[qg][po:po + DH, dc, :],
                            at_g[qg][po:po + DH, dc, :],
                            zb_q[qg][po:po + DH, h * QGW:(h + 1) * QGW],
                        )
                out.append(mul_unit)
                for nck in range(kc_n):
                    def ff_unit(qg=qg, nck=nck):
                        fp = ps.tile([P, QGW], F32, tag="pacc", bufs=2,
                                     name=f"fp_{nck}_{qg}")
                        for dc in range(dc_n):
                            nc.tensor.matmul(
                                fp[:],
                                lhsT=wff_sb[:, dc, nck * P:(nck + 1) * P],
                                rhs=at_g[qg][:, dc, :],
                                start=(dc == 0),
                                stop=(dc == dc_n - 1),
                            )
                        ot = opool.tile([P, QGW], F32, tag="otile",
                                        name=f"ot_{nck}_{qg}")
                        if use_bias:
                            nc.scalar.activation(ot[:], fp[:], AF.Identity,
                                                 bias=bffq_sb[:, nck:nck + 1])
                        else:
                            nc.vector.tensor_copy(ot[:], fp[:])
                        nc.sync.dma_start(
                            outT[nck * P:(nck + 1) * P,
                                 qg * QGW:(qg + 1) * QGW], ot[:])
                    out.append(ff_unit)

            def run_units(units, n=None):
                k = len(units) if n is None else min(n, len(units))
                for _ in range(k):
                    units.popleft()()

            def attention(qg, fillers):
                kmax = tt
                PW = 2  # score tiles batched per exp
                nquad = kmax // PW
                for h in range(hpc):
                    po = (h * DH) % P
                    dch = (h * DH) // P
                    op = ps.tile([P, QGW], F32, tag="opacc", bufs=2,
                                 name=f"op_{h}_{qg}")
                    ets = [None] * nquad

                    def emit_scores(qd):
                        sp = ps.tile([P, PW * QGW], F32, tag="mmw", bufs=2,
                                     name=f"sp_{h}_{qg}_{qd}")
                        for j in range(PW):
                            kt = qd * PW + j
                            kg, kx = kt // tpg, kt % tpg
                            kh = kT_g[kg][po:po + DH, dch,
                                          kx * P:(kx + 1) * P]
                            nc.tensor.matmul(
                                sp[:, j * QGW:(j + 1) * QGW],
                                lhsT=kh,
                                rhs=qT_g[qg][po:po + DH, dch, :],
                                start=True,
                                stop=True,
                            )
                            if variant == "generic":
                                mb_sb = xin.tile([P, QGW], F32, tag="mstream",
                                                 bufs=4,
                                                 name=f"mb_{h}_{qg}_{kt}")
                                nc.sync.dma_start(
                                    mb_sb[:],
                                    mbT[kt * P:(kt + 1) * P,
                                        qg * QGW:(qg + 1) * QGW])
                                nc.vector.tensor_add(
                                    sp[:, j * QGW:(j + 1) * QGW],
                                    sp[:, j * QGW:(j + 1) * QGW], mb_sb[:])
                        et = epool.tile([P, PW * QGW], BF16, tag="etile",
                                        name=f"et_{h}_{qg}_{qd}")
                        nc.scalar.activation(et[:], sp[:], AF.Exp)
                        ets[qd] = et

                    def emit_av(qd):
                        et = ets[qd]
                        for j in range(PW):
                            kt = qd * PW + j
                            kg, kx = kt // tpg, kt % tpg
                            nc.tensor.matmul(
                                op[:DH + 1, :],
                                lhsT=va_g[kg][:, kx, h * (DH + 1):
                                              (h + 1) * (DH + 1)],
                                rhs=et[:, j * QGW:(j + 1) * QGW],
                                start=(kt == 0),
                                stop=(kt == kmax - 1),
                            )
                        ets[qd] = None

                    emit_scores(0)
                    for qd in range(1, nquad):
                        emit_scores(qd)
                        run_units(fillers, 1)
                        emit_av(qd - 1)
                    emit_av(nquad - 1)
                    run_units(fillers, 1)
                    nc.vector.tensor_copy(
                        at_g[qg][po:po + DH, dch, :], op[:DH, :])
                    nc.vector.tensor_copy(
                        z_q[qg][0:1, h * QGW:(h + 1) * QGW],
                        op[DH:DH + 1, :])
                    hs = slice(h * QGW, (h + 1) * QGW)
                    nc.vector.reciprocal_approx_fast(zi_q[qg][0:1, hs],
                                                     z_q[qg][0:1, hs])
                    nc.sync.dma_start(zdr[qg:qg + 1, hs], zi_q[qg][0:1, hs])
                    nc.sync.dma_start(
                        zb_q[qg][:, hs],
                        zdr[qg:qg + 1, hs].to_broadcast([P, QGW]))

            from collections import deque
            fillers = deque()

            def queue_proj(tg):
                fillers.append(lambda: load_x("xq", xq_sb, xqT))
                proj_qk_units(tg, wq_sb, xq_sb, bq_sb if use_bias else None,
                              qT_g[tg], fillers)
                fillers.append(lambda: load_w("wk", wk_sb, wkT))
                fillers.append(lambda: load_x("xk", xk_sb, xkT))
                proj_qk_units(tg, wk_sb, xk_sb, bk_sb if use_bias else None,
                              kT_g[tg], fillers)
                fillers.append(lambda: load_w("wv", wv_sb, wvT))
                proj_v_units(tg, fillers)

            for tg in range(tg_n):
                queue_proj(tg)
                run_units(fillers)
            load_w("wff", wff_sb, wffT)
            for qg in range(tg_n):
                if qg > 0:
                    norm_ff_units(qg - 1, fillers)
                attention(qg, fillers)
                run_units(fillers)
            norm_ff_units(tg_n - 1, fillers)
            run_units(fillers)

    nc.compile()
    return nc


def _classify_mask(mask: np.ndarray) -> str:
    m = np.asarray(mask)[:, 0]  # [B, S, S]
    if not m.any():
        return "dense"
    s = m.shape[-1]
    causal = np.triu(np.ones((s, s), dtype=m.dtype), k=1)
    if all(np.array_equal(m[b], causal) for b in range(m.shape[0])):
        return "causal"
    return "generic"


def _bf(x):
    return np.ascontiguousarray(np.ascontiguousarray(x).astype(NPBF16))


def _make_in_maps(variant, query, key, value, mask, wq, bq, wk, bk, wv, bv,
                  wff, bff, use_bias):
    scale = np.float32(1.0 / np.sqrt(np.float32(DH)))
    if variant == "causal":
        # prepacked group-major layouts (all transforms on host, free)
        wqs = (wq * scale).T.reshape(KC_N, P, D)   # [kc, p, m_full]
        wkT = wk.T.reshape(KC_N, P, D)
        wvT = wv.T.reshape(KC_N, P, D)
        wffT = wff.T                                # [d_in=1024? no: (D, D)]

        def xpack(x, b):
            # x[b].T [D, S] -> [tg, p, kc, j]
            xt = _bf(x[b].T)
            return np.ascontiguousarray(
                xt.reshape(KC_N, P, TG_N, QGW).transpose(2, 1, 0, 3))

        xq_p = [xpack(query, b) for b in range(B)]
        xk_p = [xpack(key, b) for b in range(B)]
        xv_p = [xpack(value, b) for b in range(B)]
        dmask = np.tril(np.ones((P, P), np.float32)).T  # [k,q] 1 if k<=q

        in_maps = []
        for c in range(NCORES):
            b, hg = c // GPB, c % GPB
            sl = slice(hg * HD, (hg + 1) * HD)
            m = {
                "xq": xq_p[b], "xk": xk_p[b], "xv": xv_p[b],
                "wq": _bf(wqs[:, :, sl].transpose(1, 0, 2)),
                "wk": _bf(wkT[:, :, sl].transpose(1, 0, 2)),
                "wv": _bf(wvT[:, :, sl].transpose(1, 0, 2)),
                # wff rows for this head slice: [256, 1024] -> [p, dc, n]
                "wff": _bf(wff.T[sl, :].reshape(DC_N, P, D)
                           .transpose(1, 0, 2)),
                "dmask": _bf(dmask),
            }
            if use_bias:
                m["bq"] = np.ascontiguousarray(
                    (bq * scale)[sl].reshape(DC_N, P).T).astype(np.float32)
                m["bk"] = np.ascontiguousarray(
                    bk[sl].reshape(DC_N, P).T).astype(np.float32)
                m["bv"] = _bf(bv[sl])[None, :]
                m["bffq"] = np.ascontiguousarray(
                    (bff / GPB).reshape(KC_N, P).T).astype(np.float32)
                m["onesb"] = np.ones((1, P), NPBF16)
            in_maps.append(m)
        return in_maps

    # fallback variants (dense / generic)
    wqTs = _bf((wq * scale).T)
    wkT = _bf(wk.T)
    wvT = _bf(wv.T)
    wffT = _bf(wff.T)

    qT = [_bf(query[b].T) for b in range(B)]
    kT = [_bf(key[b].T) for b in range(B)]
    vT = [_bf(value[b].T) for b in range(B)]
    mbT = None
    if variant == "generic":
        mbT = [np.ascontiguousarray(mask[b, 0].T * np.float32(-1e9))
               for b in range(B)]

    in_maps = []
    for c in range(NCORES):
        b, hg = c // GPB, c % GPB
        sl = slice(hg * HD, (hg + 1) * HD)
        m = {
            "xqT": qT[b], "xkT": kT[b], "xvT": vT[b],
            "wqT": np.ascontiguousarray(wqTs[:, sl]),
            "wkT": np.ascontiguousarray(wkT[:, sl]),
            "wvT": np.ascontiguousarray(wvT[:, sl]),
            "wffT": np.ascontiguousarray(wffT[sl, :]),
        }
        if use_bias:
            m["bq"] = np.ascontiguousarray((bq * scale)[sl]).astype(np.float32)
            m["bk"] = np.ascontiguousarray(bk[sl]).astype(np.float32)
            m["bv"] = _bf(bv[sl])[None, :]
            m["bffq"] = (bff / GPB).astype(np.float32)
            m["onesb"] = np.ones((1, P), NPBF16)
        if variant == "generic":
            m["mbT"] = mbT[b]
        in_maps.append(m)
    return in_maps


def kernel(**inputs) -> np.ndarray:
    query = np.ascontiguousarray(inputs["query"], dtype=np.float32)
    key = np.ascontiguousarray(inputs["key"], dtype=np.float32)
    value = np.ascontiguousarray(inputs["value"], dtype=np.float32)
    mask = np.asarray(inputs["mask"], dtype=np.float32)
    wq = np.asarray(inputs["wq"], np.float32)
    bq = np.asarray(inputs["bq"], np.float32)
    wk = np.asarray(inputs["wk"], np.float32)
    bk = np.asarray(inputs["bk"], np.float32)
    wv = np.asarray(inputs["wv"], np.float32)
    bv = np.asarray(inputs["bv"], np.float32)
    wff = np.asarray(inputs["wff"], np.float32)
    bff = np.asarray(inputs["bff"], np.float32)

    variant = _classify_mask(mask)
    use_bias = bool(bq.any() or bk.any() or bv.any() or bff.any())
    pkey = (variant, use_bias)
    if pkey not in _PROG_CACHE:
        if variant == "causal":
            _PROG_CACHE[pkey] = build_causal(use_bias)
        else:
            _PROG_CACHE[pkey] = build_program(variant, use_bias)
    nc = _PROG_CACHE[pkey]

    in_maps = _make_in_maps(variant, query, key, value, mask, wq, bq, wk, bk,
                            wv, bv, wff, bff, use_bias)
    res = run_bass_kernel_spmd(nc, in_maps, core_ids=list(range(NCORES)))
    out = np.empty((B, S, D), np.float32)
    for b in range(B):
        acc = res.results[b * GPB]["outT"].astype(np.float32)
        for g in range(1, GPB):
            acc = acc + res.results[b * GPB + g]["outT"].astype(np.float32)
        out[b] = acc.T
    return out


if __name__ == "__main__":
    import reference

    inputs = {k: np.asarray(v) for k, v in reference.setup_inputs().items()}
    out = kernel(**inputs)
    print("kernel out:", out.shape, out.dtype)
